# revision 20
# baseline (speedup 1.0000x reference)
"""LoRALinear kernel for Trainium2 (8 NeuronCores, data-parallel over tokens).

Math: out = x @ W.T + b + s1*(x@A1.T)@B1.T + s2*(x@A2.T)@B2.T
         = x @ W'.T + b,   W' = W + s1*B1@A1 + s2*B2@A2

Strategy: data-parallel shard x (4096 tokens/core), replicate weights.
The dense [4096,1024]@[1024,1024] matmul per core runs on the PE array in
fp8 (e4m3) DoubleRow perf mode (two 128-deep k-tiles per pass), using a
3-stream residual-compensated quantization so accuracy stays ~2e-3:

  x  ~ x1/32 + x2/512            (x1 = e4m3(32*x), x2 = e4m3(512*(x - x1/32)))
  W' ~ W1/512 + w2/16384         (W1 = e4m3(512*W'), w2 = e4m3(512*W' - W1))
  out*2^14 = x1@W1 + x2@W1b + x1@w2     (W1b = e4m3(32*W') pairs with x2 so
                                         every product shares scale 2^14)

W (bf16, x512) is DMA'd; the rank-32 LoRA fold runs as 16 fp32r PE matmuls;
V = 512*W'.T accumulates in bf16, then DVE quantizes V into W1/W1b/w2 in
per-of-tile chains pipelined ahead of the PE main loop.

Main loop is transposed (out-features on PSUM partitions) so the Activation
engine applies `out = psum * 2^-14 + b[of]` as one fused per-partition op,
and fp32 results DMA out in contiguous rows of outT[of, tok]. Host packs x
into the fp8 SBUF layout and transposes outT back at the end.
"""

import sys

import numpy as np
import ml_dtypes

try:
    import concourse.bass as bass  # noqa: F401
except ImportError:
    sys.path.insert(0, "/opt/trn_rl_repo")
    import concourse.bass as bass  # noqa: F401

from concourse import bacc

import concourse.mybir as mybir
import concourse.tile as tile
from concourse.bass_utils import run_bass_kernel_spmd

TOKENS, D, RANK = 32768, 1024, 16
N_CORES = 8
T_SHARD = TOKENS // N_CORES  # 4096
SCALE1 = 8.0 / RANK
SCALE2 = 16.0 / RANK
F32 = mybir.dt.float32
F32R = mybir.dt.float32r
BF16 = mybir.dt.bfloat16
FP8 = mybir.dt.float8e4
E4NP = ml_dtypes.float8_e4m3
P = 128
NIO = D // P  # 8 k-blocks of 128
NG = NIO // 2  # 4 DoubleRow k-groups of 256
N_OT = D // P  # 8 out-feature tiles of 128
TCW = 512  # token chunk width
N_TC = T_SHARD // TCW  # 8
SX1 = 32.0  # x stream-1 scale
SX2 = 512.0  # x residual scale
SW = 512.0  # W scale; W1b at SW/16
PSCALE = 1.0 / (SX1 * SW)  # 2^-14, exact
DR = mybir.MatmulPerfMode.DoubleRow
IDENT = mybir.ActivationFunctionType.Identity


def build_nc():
    nc = bacc.Bacc("TRN2")
    x1 = nc.dram_tensor("x1", [P, NIO, T_SHARD], FP8, kind="ExternalInput")
    x2 = nc.dram_tensor("x2", [P, NIO, T_SHARD], FP8, kind="ExternalInput")
    Wp = nc.dram_tensor("Wp", [P, NIO, D], BF16, kind="ExternalInput")
    Acat = nc.dram_tensor("Acat", [2 * RANK, D], BF16, kind="ExternalInput")
    Bcat = nc.dram_tensor("Bcat", [2 * RANK, D], BF16, kind="ExternalInput")
    bvec = nc.dram_tensor("bvec", [P, N_OT], F32, kind="ExternalInput")
    outT = nc.dram_tensor("outT", [D, T_SHARD], F32, kind="ExternalOutput")

    with tile.TileContext(nc) as tc:
        with (
            tc.tile_pool(name="const", bufs=1) as const,
            tc.tile_pool(name="op", bufs=18) as opool,
            tc.tile_pool(name="psf", bufs=4, space="PSUM") as psumf,
            tc.tile_pool(name="psm", bufs=4, space="PSUM") as psum,
        ):
            # --- prep DMAs; SP issues in-order so order = priority.
            # W arrives in of-half chunks: the first half feeds the on=0
            # V-adds while x-e0/e1 and the second half stream behind it. ---
            A_ld = const.tile([2 * RANK, D], BF16)
            nc.sync.dma_start(A_ld, Acat[:])
            B_ld = const.tile([2 * RANK, D], BF16)
            nc.sync.dma_start(B_ld, Bcat[:])
            Wld = const.tile([P, NIO, D], BF16)
            for io in range(NIO):
                nc.sync.dma_start(Wld[:, io, 0:512], Wp[:, io, 0:512])
            x1_sb = const.tile([P, NIO, T_SHARD], FP8)
            x2_sb = const.tile([P, NIO, T_SHARD], FP8)
            for e in range(2):
                sl = slice(e * TCW, (e + 1) * TCW)
                nc.sync.dma_start(x1_sb[:, :, sl], x1[:, :, sl])
                nc.sync.dma_start(x2_sb[:, :, sl], x2[:, :, sl])
            bias_sb = const.tile([P, N_OT], F32)
            nc.sync.dma_start(bias_sb, bvec[:])
            for io in range(NIO):
                nc.sync.dma_start(Wld[:, io, 512:1024], Wp[:, io, 512:1024])
            for e in range(2, N_TC):
                sl = slice(e * TCW, (e + 1) * TCW)
                nc.sync.dma_start(x1_sb[:, :, sl], x1[:, :, sl])
                nc.sync.dma_start(x2_sb[:, :, sl], x2[:, :, sl])

            # --- adapter operands: bf16 (B pre-scaled on host) ---
            A_sb = A_ld
            Bs_sb = B_ld

            # --- fold V = 512*W'.T (bf16), quantize to W1/W1b/w2 (fp8) ---
            # V-adds split across DVE (on=0) and GpSimd (on=1) so the per-ot
            # quant chains (W1/w2 on DVE, W1b on Act) can run back-to-back on
            # DVE and feed the main loop at a ~1.5us/ot cadence.
            V = const.tile([P, NIO, D], BF16)
            W1 = const.tile([P, NIO, D], FP8)
            W1b = const.tile([P, NIO, D], FP8)
            w2 = const.tile([P, NIO, D], FP8)

            def emit_fold(on, ics, with_adds=True):
                # GpSimd cannot read PSUM on TRN2, so every V-add is on DVE.
                osl = slice(on * 512, (on + 1) * 512)
                psfs = []
                for ic in ics:
                    psf = psumf.tile([P, 512], F32, tag="psf")
                    nc.tensor.matmul(
                        psf,
                        lhsT=A_sb[:, ic * P : (ic + 1) * P],
                        rhs=Bs_sb[:, osl],
                        start=True,
                        stop=True,
                    )
                    if with_adds:
                        nc.vector.tensor_add(
                            out=V[:, ic, osl], in0=psf, in1=Wld[:, ic, osl]
                        )
                    else:
                        psfs.append((ic, psf))
                return psfs

            def emit_adds(on, psfs):
                osl = slice(on * 512, (on + 1) * 512)
                for ic, psf in psfs:
                    nc.vector.tensor_add(out=V[:, ic, osl], in0=psf, in1=Wld[:, ic, osl])

            def emit_quants(ots):
                for ot in ots:
                    otsl = slice(ot * P, (ot + 1) * P)
                    nc.vector.tensor_copy(out=W1[:, :, otsl], in_=V[:, :, otsl])
                    nc.vector.tensor_sub(w2[:, :, otsl], V[:, :, otsl], W1[:, :, otsl])

            def emit_w1b(ots):
                for ot in ots:
                    otsl = slice(ot * P, (ot + 1) * P)
                    nc.scalar.mul(W1b[:, :, otsl], V[:, :, otsl], 1.0 / 16.0)

            emit_fold(0, range(NIO))
            psfs_1a = emit_fold(1, range(0, 4), with_adds=False)
            emit_quants(range(0, 4))
            emit_w1b(range(0, 4))
            emit_adds(1, psfs_1a)

            # --- main loop: phased traversal keeps PE fed by both the W-quant
            # pipeline (by ot) and the x DMA stream (by tc); out DMA per cell ---
            streams = ((x1_sb, W1), (x1_sb, w2), (x2_sb, W1b))

            def emit_cell(ot, t0, tw):
                otsl = slice(ot * P, (ot + 1) * P)
                tsl = slice(t0, t0 + tw)
                ps = psum.tile([P, TCW], F32, tag="ps")
                k = 0
                for xs, ws in streams:
                    for g in range(NG):
                        gsl = slice(2 * g, 2 * g + 2)
                        nc.tensor.matmul(
                            ps[:, 0:tw],
                            lhsT=ws[:, gsl, otsl],
                            rhs=xs[:, gsl, tsl],
                            start=(k == 0),
                            stop=(k == 11),
                            perf_mode=DR,
                        )
                        k += 1
                o_sb = opool.tile([P, TCW], F32, tag="o")
                nc.scalar.activation(
                    o_sb[:, 0:tw],
                    ps[:, 0:tw],
                    IDENT,
                    bias=bias_sb[:, ot : ot + 1],
                    scale=PSCALE,
                )
                nc.sync.dma_start(outT[otsl, tsl], o_sb[:, 0:tw])

            # phase 1: first three token columns over the ready half (ot0-3),
            # while the second fold wave + quants run underneath
            for tcc in range(3):
                for ot in range(0, 4):
                    emit_cell(ot, tcc * TCW, TCW)
            psfs_1b = emit_fold(1, range(4, 8), with_adds=False)
            emit_adds(1, psfs_1b)
            emit_quants(range(4, 8))
            emit_w1b(range(4, 8))
            for tcc in range(3, 5):
                for ot in range(0, 4):
                    emit_cell(ot, tcc * TCW, TCW)
            for tcc in range(5):
                for ot in range(4, 8):
                    emit_cell(ot, tcc * TCW, TCW)
            for tcc in range(5, N_TC - 1):
                for ot in range(N_OT):
                    emit_cell(ot, tcc * TCW, TCW)
            # last column in half-width cells for a shorter drain tail
            for half in range(2):
                for ot in range(N_OT):
                    emit_cell(ot, (N_TC - 1) * TCW + half * (TCW // 2), TCW // 2)

    nc.finalize()
    return nc


_NC = None


def _get_nc():
    global _NC
    if _NC is None:
        _NC = build_nc()
    return _NC


def _pack_x(q, c):
    # [T_SHARD, D] fp8 -> [P, NIO, T_SHARD] with [p, io, t] = q[t, io*128+p]
    blk = q[c * T_SHARD : (c + 1) * T_SHARD]
    return np.ascontiguousarray(blk.reshape(T_SHARD, NIO, P).transpose(2, 1, 0))


def kernel(**inputs):
    x = np.asarray(inputs["x"], np.float32)
    W = np.asarray(inputs["W"], np.float32)
    b = np.asarray(inputs["b"], np.float32)
    A1 = np.asarray(inputs["A1"], np.float32)
    B1 = np.asarray(inputs["B1"], np.float32)
    A2 = np.asarray(inputs["A2"], np.float32)
    B2 = np.asarray(inputs["B2"], np.float32)

    # dual-stream fp8 encoding of x (power-of-2 scales)
    x1q = (x * SX1).astype(E4NP)
    r = x - x1q.astype(np.float32) * (1.0 / SX1)
    x2q = (r * SX2).astype(E4NP)

    Wp = (W.T * SW).astype(ml_dtypes.bfloat16)  # [k, o] scaled
    Wp = np.ascontiguousarray(Wp.reshape(NIO, P, D).transpose(1, 0, 2))
    shared = {
        "Wp": Wp,
        "Acat": np.ascontiguousarray(
            np.concatenate([A1, A2], axis=0).astype(ml_dtypes.bfloat16)
        ),
        "Bcat": np.ascontiguousarray(
            np.concatenate([B1.T * (SCALE1 * SW), B2.T * (SCALE2 * SW)], axis=0)
            .astype(ml_dtypes.bfloat16)
        ),
        "bvec": np.ascontiguousarray(b.reshape(N_OT, P).T),
    }
    in_maps = []
    for c in range(N_CORES):
        m = dict(shared)
        m["x1"] = _pack_x(x1q, c)
        m["x2"] = _pack_x(x2q, c)
        in_maps.append(m)
    res = run_bass_kernel_spmd(_get_nc(), in_maps, core_ids=list(range(N_CORES)))
    return np.concatenate(
        [np.asarray(r_["outT"]).T for r_ in res.results], axis=0
    ).astype(np.float32, copy=False)


# revision 35
# speedup vs baseline: 1.0053x; 1.0053x over previous
"""LoRALinear kernel for Trainium2 (8 NeuronCores, data-parallel over tokens).

Math: out = x @ W.T + b + s1*(x@A1.T)@B1.T + s2*(x@A2.T)@B2.T
         = x @ W'.T + b,   W' = W + s1*B1@A1 + s2*B2@A2

Strategy: data-parallel shard x (4096 tokens/core), replicate weights.
The dense [4096,1024]@[1024,1024] matmul per core runs on the PE array in
fp8 (e4m3) DoubleRow perf mode (two 128-deep k-tiles per pass), using a
3-stream residual-compensated quantization so accuracy stays ~2e-3:

  x  ~ x1/32 + x2/512            (x1 = e4m3(32*x), x2 = e4m3(512*(x - x1/32)))
  W' ~ W1/512 + w2/16384         (W1 = e4m3(512*W'), w2 = e4m3(512*W' - W1))
  out*2^14 = x1@W1 + x2@W1b + x1@w2     (W1b = e4m3(32*W') pairs with x2 so
                                         every product shares scale 2^14)

W (bf16, x512) is DMA'd; the rank-32 LoRA fold runs as 16 bf16 PE matmuls;
V = 512*W'.T accumulates in bf16, then DVE quantizes V into W1/W1b/w2 in
per-of-tile chains pipelined ahead of the PE main loop.

Main loop is transposed (out-features on PSUM partitions) so the Activation
engine applies `out = psum * 2^-14 + b[of]` as one fused per-partition op,
and fp32 results DMA out in contiguous rows of outT[of, tok]. Host packs x
into the fp8 SBUF layout and transposes outT back at the end.
"""

import sys

import numpy as np
import ml_dtypes

try:
    import concourse.bass as bass  # noqa: F401
except ImportError:
    sys.path.insert(0, "/opt/trn_rl_repo")
    import concourse.bass as bass  # noqa: F401

from concourse import bacc

import concourse.mybir as mybir
import concourse.tile as tile
from concourse.bass_utils import run_bass_kernel_spmd

TOKENS, D, RANK = 32768, 1024, 16
N_CORES = 8
T_SHARD = TOKENS // N_CORES  # 4096
SCALE1 = 8.0 / RANK
SCALE2 = 16.0 / RANK
F32 = mybir.dt.float32
BF16 = mybir.dt.bfloat16
FP8 = mybir.dt.float8e4
E4NP = ml_dtypes.float8_e4m3
P = 128
NIO = D // P  # 8 k-blocks of 128
NG = NIO // 2  # 4 DoubleRow k-groups of 256
N_OT = D // P  # 8 out-feature tiles of 128
TCW = 512  # token chunk width
N_TC = T_SHARD // TCW  # 8
SX1 = 32.0  # x stream-1 scale
SX2 = 512.0  # x residual scale
SW = 512.0  # W scale; W1b at SW/16
PSCALE = 1.0 / (SX1 * SW)  # 2^-14, exact
DR = mybir.MatmulPerfMode.DoubleRow
IDENT = mybir.ActivationFunctionType.Identity


def build_nc():
    nc = bacc.Bacc("TRN2")
    x1 = nc.dram_tensor("x1", [P, NIO, T_SHARD], FP8, kind="ExternalInput")
    x2 = nc.dram_tensor("x2", [P, NIO, T_SHARD], FP8, kind="ExternalInput")
    Wp = nc.dram_tensor("Wp", [P, NIO, D], BF16, kind="ExternalInput")
    Acat = nc.dram_tensor("Acat", [2 * RANK, D], BF16, kind="ExternalInput")
    Bcat = nc.dram_tensor("Bcat", [2 * RANK, D], BF16, kind="ExternalInput")
    bvec = nc.dram_tensor("bvec", [P, N_OT], F32, kind="ExternalInput")
    outT = nc.dram_tensor("outT", [D, T_SHARD], F32, kind="ExternalOutput")

    with tile.TileContext(nc) as tc:
        with (
            tc.tile_pool(name="const", bufs=1) as const,
            tc.tile_pool(name="op", bufs=18) as opool,
            tc.tile_pool(name="psf", bufs=3, space="PSUM") as psumf,
            tc.tile_pool(name="psm", bufs=5, space="PSUM") as psum,
        ):
            # --- prep DMAs; SP issues in-order so order = priority.
            # W arrives in of-half chunks: the first half feeds the on=0
            # V-adds while x-e0/e1 and the second half stream behind it. ---
            A_ld = const.tile([2 * RANK, D], BF16)
            nc.sync.dma_start(A_ld, Acat[:])
            B_ld = const.tile([2 * RANK, D], BF16)
            nc.sync.dma_start(B_ld, Bcat[:])
            Wld = const.tile([P, NIO, D], BF16)
            for io in range(NIO):
                nc.sync.dma_start(Wld[:, io, 0:512], Wp[:, io, 0:512])
            x1_sb = const.tile([P, NIO, T_SHARD], FP8)
            x2_sb = const.tile([P, NIO, T_SHARD], FP8)
            for e in range(2):
                sl = slice(e * TCW, (e + 1) * TCW)
                nc.sync.dma_start(x1_sb[:, :, sl], x1[:, :, sl])
                nc.sync.dma_start(x2_sb[:, :, sl], x2[:, :, sl])
            bias_sb = const.tile([P, N_OT], F32)
            nc.sync.dma_start(bias_sb, bvec[:])
            for io in range(NIO):
                nc.sync.dma_start(Wld[:, io, 512:1024], Wp[:, io, 512:1024])
            for e in range(2, N_TC):
                sl = slice(e * TCW, (e + 1) * TCW)
                nc.sync.dma_start(x1_sb[:, :, sl], x1[:, :, sl])
                nc.sync.dma_start(x2_sb[:, :, sl], x2[:, :, sl])

            # --- adapter operands: bf16 (B pre-scaled on host) ---
            A_sb = A_ld
            Bs_sb = B_ld

            # --- fold V = 512*W'.T (bf16), quantize to W1/W1b/w2 (fp8) ---
            # The second fold wave is split in two so its psum tiles never
            # collide with the main loop's, and the DVE stream runs
            # [adds-on0, quants ot0-3, adds-on1a, ..., adds-on1b, quants
            # ot4-7] to unblock of-tiles just ahead of the PE phases.
            V = const.tile([P, NIO, D], BF16)
            W1 = const.tile([P, NIO, D], FP8)
            W1b = const.tile([P, NIO, D], FP8)
            w2 = const.tile([P, NIO, D], FP8)

            def emit_fold(on, ics, with_adds=True):
                # GpSimd cannot read PSUM on TRN2, so every V-add is on DVE.
                osl = slice(on * 512, (on + 1) * 512)
                psfs = []
                for ic in ics:
                    psf = psumf.tile([P, 512], F32, tag="psf")
                    nc.tensor.matmul(
                        psf,
                        lhsT=A_sb[:, ic * P : (ic + 1) * P],
                        rhs=Bs_sb[:, osl],
                        start=True,
                        stop=True,
                    )
                    if with_adds:
                        nc.vector.tensor_add(
                            out=V[:, ic, osl], in0=psf, in1=Wld[:, ic, osl]
                        )
                    else:
                        psfs.append((ic, psf))
                return psfs

            def emit_adds(on, psfs):
                osl = slice(on * 512, (on + 1) * 512)
                for ic, psf in psfs:
                    nc.vector.tensor_add(out=V[:, ic, osl], in0=psf, in1=Wld[:, ic, osl])

            def emit_quants(ots):
                for ot in ots:
                    otsl = slice(ot * P, (ot + 1) * P)
                    nc.vector.tensor_copy(out=W1[:, :, otsl], in_=V[:, :, otsl])
                    nc.vector.tensor_sub(w2[:, :, otsl], V[:, :, otsl], W1[:, :, otsl])

            def emit_w1b(ots):
                for ot in ots:
                    otsl = slice(ot * P, (ot + 1) * P)
                    nc.scalar.mul(W1b[:, :, otsl], V[:, :, otsl], 1.0 / 16.0)

            emit_fold(0, range(NIO))
            psfs_1a = emit_fold(1, range(0, 4), with_adds=False)
            emit_quants(range(0, 4))
            emit_w1b(range(0, 4))
            emit_adds(1, psfs_1a)

            # --- main loop: phased traversal keeps PE fed by both the W-quant
            # pipeline (by ot) and the x DMA stream (by tc); out DMA per cell ---
            streams = ((x1_sb, W1), (x2_sb, W1b), (x1_sb, w2))

            def emit_cell(ot, t0, tw):
                otsl = slice(ot * P, (ot + 1) * P)
                tsl = slice(t0, t0 + tw)
                ps = psum.tile([P, TCW], F32, tag="ps")
                k = 0
                for xs, ws in streams:
                    for g in range(NG):
                        gsl = slice(2 * g, 2 * g + 2)
                        nc.tensor.matmul(
                            ps[:, 0:tw],
                            lhsT=ws[:, gsl, otsl],
                            rhs=xs[:, gsl, tsl],
                            start=(k == 0),
                            stop=(k == 11),
                            perf_mode=DR,
                        )
                        k += 1
                o_sb = opool.tile([P, TCW], F32, tag="o")
                nc.scalar.activation(
                    o_sb[:, 0:tw],
                    ps[:, 0:tw],
                    IDENT,
                    bias=bias_sb[:, ot : ot + 1],
                    scale=PSCALE,
                )
                nc.sync.dma_start(outT[otsl, tsl], o_sb[:, 0:tw])

            # phase 1: first three token columns over the ready half (ot0-3),
            # while the second fold wave + quants run underneath
            for tcc in range(3):
                for ot in range(0, 4):
                    emit_cell(ot, tcc * TCW, TCW)
            psfs_1b = emit_fold(1, range(4, 8), with_adds=False)
            emit_adds(1, psfs_1b)
            emit_quants(range(4, 8))
            emit_w1b(range(4, 8))
            for tcc in range(3, 5):
                for ot in range(0, 4):
                    emit_cell(ot, tcc * TCW, TCW)
            for tcc in range(5):
                for ot in range(4, 8):
                    emit_cell(ot, tcc * TCW, TCW)
            for tcc in range(5, N_TC - 1):
                for ot in range(N_OT):
                    emit_cell(ot, tcc * TCW, TCW)
            # last column in half-width cells for a shorter drain tail
            for half in range(2):
                for ot in range(N_OT):
                    emit_cell(ot, (N_TC - 1) * TCW + half * (TCW // 2), TCW // 2)

    nc.finalize()
    return nc


_NC = None


def _get_nc():
    global _NC
    if _NC is None:
        _NC = build_nc()
    return _NC


def _pack_x(q, c):
    # [T_SHARD, D] fp8 -> [P, NIO, T_SHARD] with [p, io, t] = q[t, io*128+p]
    blk = q[c * T_SHARD : (c + 1) * T_SHARD]
    return np.ascontiguousarray(blk.reshape(T_SHARD, NIO, P).transpose(2, 1, 0))


def kernel(**inputs):
    x = np.asarray(inputs["x"], np.float32)
    W = np.asarray(inputs["W"], np.float32)
    b = np.asarray(inputs["b"], np.float32)
    A1 = np.asarray(inputs["A1"], np.float32)
    B1 = np.asarray(inputs["B1"], np.float32)
    A2 = np.asarray(inputs["A2"], np.float32)
    B2 = np.asarray(inputs["B2"], np.float32)

    # dual-stream fp8 encoding of x (power-of-2 scales)
    x1q = (x * SX1).astype(E4NP)
    r = x - x1q.astype(np.float32) * (1.0 / SX1)
    x2q = (r * SX2).astype(E4NP)

    Wp = (W.T * SW).astype(ml_dtypes.bfloat16)  # [k, o] scaled
    Wp = np.ascontiguousarray(Wp.reshape(NIO, P, D).transpose(1, 0, 2))
    shared = {
        "Wp": Wp,
        "Acat": np.ascontiguousarray(
            np.concatenate([A1, A2], axis=0).astype(ml_dtypes.bfloat16)
        ),
        "Bcat": np.ascontiguousarray(
            np.concatenate([B1.T * (SCALE1 * SW), B2.T * (SCALE2 * SW)], axis=0)
            .astype(ml_dtypes.bfloat16)
        ),
        "bvec": np.ascontiguousarray(b.reshape(N_OT, P).T),
    }
    in_maps = []
    for c in range(N_CORES):
        m = dict(shared)
        m["x1"] = _pack_x(x1q, c)
        m["x2"] = _pack_x(x2q, c)
        in_maps.append(m)
    res = run_bass_kernel_spmd(_get_nc(), in_maps, core_ids=list(range(N_CORES)))
    return np.concatenate(
        [np.asarray(r_["outT"]).T for r_ in res.results], axis=0
    ).astype(np.float32, copy=False)
